# revision 23
# baseline (speedup 1.0000x reference)
"""Multi-head attention, tensor-parallel across 8 Trainium2 NeuronCores.

Sharding: core = (batch b, head-group g), g covering 4 heads (256 dh).
Within a core heads are processed as PAIRS: head A of a pair lives on
SBUF partitions 0-63, head B on 64-127, so the two K=64 scores matmuls
run concurrently on 64x128-mode array tiles T0/T8.

The attention is one flat software pipeline over (i-block, pair, j-tile)
steps batched two at a time (scores+exp of steps g, g+1 then the AV
matmuls of steps g-2, g-1), which keeps the exp pipeline primed across
i-block boundaries.  The output projection is interleaved into the
attention stream: chunk i's matmuls are emitted a few steps into block
i+1 (by which time both head-pairs of block i are normalized), filling
PE bubbles instead of running as a serial tail phase.

exp is split between ScalarE (exact Exp) and VectorE (a round-to-
nearest int16 Schraudolph in one tensor_scalar): bf16bits(exp(x)) ~
round(x*A + B) (fp32->int16 convert verified round-to-nearest on HW;
tile is bitcast to bf16 when consumed). Its ratio error is calibrated
mean-one, and softmax normalization cancels the common mode, ~1.3%.

Z comes from a ones column appended to V (AV psum row 64).  The Z rows
are reciprocal'd in-lane (1-lane ACT/DVE ops, no DRAM round trips),
broadcast across partitions by GPSIMD partition_broadcast, and GPSIMD
does the normalize mults.  Head B's normalized block is staged and
DMA'd to partitions 64-127 on the scalar DGE queue so it never queues
behind bulk output stores.  The last block runs its whole chain on
ACT/DVE at high priority to minimize the end-of-kernel latency.

DMA uses both hardware DGE queues: weights + latency-critical small
transfers on the Activation queue, bulk streams + output stores on SP.

Host: shards inputs, sums the 4 head-group partials per batch, adds bo.
"""

import os
import numpy as np

DBG_EXP_ACT = os.environ.get("DBG_EXP_ACT", "0") == "1"

B, S, D, H = 2, 2048, 1024, 16
DK = D // H              # 64 head dim
N_CORES = 8
GROUPS = N_CORES // B    # 4 head-groups
DH = D // GROUPS         # 256 head-dims per core (4 heads)
H_CORE = DH // DK        # 4 heads per core
SCALE = 1.0 / float(np.sqrt(DK))

P = 128                  # SBUF/PSUM partitions
SC = 512                 # matmul moving-dim chunk
IB = 512                 # flash i-block
LOG2E = float(np.log2(np.e))
SCH_A = float(128.0 * SCALE * LOG2E)       # schraudolph slope
SCH_B = float(127.0 * 128.0 - 7.35)        # schraudolph bias (mean-one)


def build_nc(S=S, D=D, DH=DH, DK=DK, scale=SCALE, ib=IB):
    import concourse.bacc as bacc
    import concourse.mybir as mybir
    import concourse.tile as tile

    f32 = mybir.dt.float32
    bf16 = mybir.dt.bfloat16
    i16 = mybir.dt.int16
    Exp = mybir.ActivationFunctionType.Exp
    Ident = mybir.ActivationFunctionType.Identity
    Recip = mybir.ActivationFunctionType.Reciprocal
    Mult = mybir.AluOpType.mult
    Add = mybir.AluOpType.add
    cdt = bf16

    KT = D // P                    # contraction tiles for projections (8)
    NSC = S // SC                  # s chunks (4)
    HC = DH // P                   # head pairs (2)
    HPC = P // DK                  # heads per pair (2)
    JT = S // P                    # j tiles (16)
    NIB = S // ib                  # i blocks (4)
    NOUT = D // P                  # output row chunks (8)
    LAG = 4                        # AV trails scores by LAG j-steps
                                   # (exp latency ~1.6us = 2.5 steps)

    nc = bacc.Bacc("TRN2", target_bir_lowering=False, debug=False)

    qT = nc.dram_tensor("qT", [D, S], cdt, kind="ExternalInput")
    kTd = nc.dram_tensor("kTd", [D, S], cdt, kind="ExternalInput")
    vT = nc.dram_tensor("vT", [D, S], cdt, kind="ExternalInput")
    wq = nc.dram_tensor("wq", [D, DH], cdt, kind="ExternalInput")
    wk = nc.dram_tensor("wk", [D, DH], cdt, kind="ExternalInput")
    wv = nc.dram_tensor("wv", [D, DH], cdt, kind="ExternalInput")
    wo = nc.dram_tensor("wo", [DH, D], cdt, kind="ExternalInput")
    bq = nc.dram_tensor("bq", [P, HC], f32, kind="ExternalInput")
    bk = nc.dram_tensor("bk", [P, HC], f32, kind="ExternalInput")
    outT = nc.dram_tensor("outT", [D, S], cdt, kind="ExternalOutput")

    with tile.TileContext(nc) as tc:
        with (
            tc.tile_pool(name="const", bufs=1) as cpool,
            tc.tile_pool(name="pers", bufs=1) as pers,
            tc.tile_pool(name="stream", bufs=1) as stream,
            tc.tile_pool(name="psum", bufs=1, space="PSUM") as psum,
            tc.tile_pool(name="dscratch", bufs=1, space="DRAM") as dscratch,
        ):
            # ---- constants ----
            wq_sb = cpool.tile([P, KT, DH], cdt, name="wq_sb")
            wk_sb = cpool.tile([P, KT, DH], cdt, name="wk_sb")
            wv_sb = cpool.tile([P, KT, DH], cdt, name="wv_sb")
            wo_sb = cpool.tile([P, HC, D], cdt, name="wo_sb")
            bq_sb = cpool.tile([P, HC], f32, name="bq_sb")
            bk_sb = cpool.tile([P, HC], f32, name="bk_sb")

            # ---- persistent activations (head-pair layout) ----
            # qt/kt pair c: rows 0-63 = head 2c (dk dims), rows 64-127 =
            # head 2c+1. v pair c: rows = j within tile, + ones column.
            qt = [pers.tile([P, S], cdt, name=f"qt{c}") for c in range(HC)]
            kt = [pers.tile([P, S], cdt, name=f"kt{c}") for c in range(HC)]
            v_c = [pers.tile([P, JT, HPC, DK + 1], cdt, name=f"v{c}")
                   for c in range(HC)]
            on_c = [pers.tile([P, S], cdt, name=f"on{c}") for c in range(HC)]

            for c in range(HC):
                nc.vector.memset(v_c[c][:, :, :, DK:DK + 1], 1.0)

            # ---- weights on the Activation DGE queue (parallel with the
            # bulk input streams on the SP queue) ----
            nc.scalar.dma_start(wq_sb[:],
                                wq[:, :].rearrange("(ko p) n -> p ko n", p=P))
            nc.scalar.dma_start(bq_sb[:], bq[:, :])
            nc.scalar.dma_start(wk_sb[:],
                                wk[:, :].rearrange("(ko p) n -> p ko n", p=P))
            nc.scalar.dma_start(bk_sb[:], bk[:, :])
            nc.scalar.dma_start(wv_sb[:],
                                wv[:, :].rearrange("(ko p) n -> p ko n", p=P))
            nc.scalar.dma_start(wo_sb[:],
                                wo[:, :].rearrange("(c p) n -> p c n", p=P))

            # ---- PE warmup: keep HAM busy during the initial input DMA
            # wait so the first real matmuls run at full clock ----
            wmp = psum.tile([P, 2 * SC], f32, tag="av", bufs=1, name="warm")
            wsrc = cpool.tile([P, DK], cdt, name="wsrc")
            nc.vector.memset(wsrc[:], 1.0)
            for w in range(64):
                nc.tensor.matmul(wmp[0:HPC, 0:DK], lhsT=wsrc[:, 0:HPC],
                                 rhs=wsrc[:, 0:DK], start=True, stop=True)

            # ---- bulk input streams.  HBM bandwidth caps total load time;
            # q's halves ride both DGE queues so the Sync queue reaches
            # kin sooner (kin completion gates attention start). ----
            def load_tensor(src, chunks, queues):
                bt = stream.tile([P, KT, S], cdt, tag="big_in", bufs=2,
                                 name=f"bi_{src.name}")
                lo = 0
                for w, dq in zip(chunks, queues):
                    hs = slice(lo, lo + w)
                    for kti in range(KT):
                        dq.dma_start(bt[:, kti, hs],
                                     src[kti * P:(kti + 1) * P, hs])
                    lo += w
                return bt

            qin = load_tensor(qT, (S // 2, S // 2), (nc.sync, nc.scalar))
            kin = load_tensor(kTd, (S,), (nc.sync,))
            vin = load_tensor(vT, (S,), (nc.sync,))

            def qk_proj(bt, w_sb, b_sb, dst):
                for si in range(NSC):
                    ps = psum.tile([P, 2 * SC], f32, tag="sc", bufs=3,
                                   name=f"ps_{dst[0].name}_{si}")
                    ssl = slice(si * SC, (si + 1) * SC)
                    for c in range(HC):
                        for kti in range(KT):
                            nc.tensor.matmul(
                                ps[:, c * SC:(c + 1) * SC],
                                lhsT=w_sb[:, kti, c * P:(c + 1) * P],
                                rhs=bt[:, kti, ssl],
                                start=(kti == 0), stop=(kti == KT - 1))
                    # evac + bias: head-pair chunk c goes straight to dst[c]
                    nc.vector.tensor_add(
                        dst[0][:, ssl], ps[:, 0:SC],
                        b_sb[:, 0:1].to_broadcast((P, SC)))
                    nc.scalar.activation(
                        dst[1][:, ssl], ps[:, SC:2 * SC], Ident,
                        bias=b_sb[:, 1:2], scale=1.0)

            qk_proj(qin, wq_sb, bq_sb, qt)
            qk_proj(kin, wk_sb, bk_sb, kt)

            # ---- V projection (natural [j, dh]), emitted interleaved
            # into early attention as quads of 4 j-tiles sharing one
            # sc-ring psum tile, so exp work starts right after K-proj
            # instead of behind a serial V phase.  bv is folded into the
            # host-side bias add (bv @ Wo), so evacs are plain copies. ----
            def vproj_quad(qd):
                ps = psum.tile([P, 2 * SC], f32, tag="sc", bufs=3,
                               name=f"ps_v_{qd}")
                for sub in range(4):
                    jt_idx = qd * 4 + sub
                    jsl = slice(jt_idx * P, (jt_idx + 1) * P)
                    for kti in range(KT):
                        nc.tensor.matmul(
                            ps[:, sub * DH:(sub + 1) * DH],
                            lhsT=vin[:, kti, jsl],
                            rhs=wv_sb[:, kti, :],
                            start=(kti == 0), stop=(kti == KT - 1))
                for sub in range(4):
                    jt_idx = qd * 4 + sub
                    for c in range(HC):
                        src_ap = ps[:, sub * DH + c * P:
                                    sub * DH + (c + 1) * P].rearrange(
                            "p (h d) -> p h d", d=DK)
                        dst_ap = v_c[c][:, jt_idx, :, 0:DK]
                        if (sub + c) % 2 == 0:
                            nc.vector.tensor_copy(dst_ap, src_ap)
                        else:
                            nc.scalar.copy(dst_ap, src_ap)

            # ---- attention (flash over j; head pairs on T0/T8) with the
            # output projection interleaved chunk-by-chunk ----
            def emit_norm(c, ibx, av):
                last = (ibx == NIB - 1 and c == HC - 1)
                isl = slice(ibx * ib, ibx * ib + ib)
                av_sb = stream.tile([P, 2 * SC], f32, tag="avsb", bufs=2,
                                    name=f"avsb_{c}_{ibx}")
                # evacuate both halves (frees the psum bank; av bufs=1)
                with tc.high_priority():
                    nc.scalar.copy(av_sb[0:DK + 1, 0:SC],
                                   av[0:DK + 1, 0:SC])
                    nc.vector.tensor_copy(av_sb[0:DK + 1, SC:2 * SC],
                                          av[0:DK + 1, SC:2 * SC])
                # Z rows DMA through DRAM reshaped [128, 8] for a cheap
                # all-lane reciprocal (1-lane DVE ops are ~6.5us - never).
                zw = (2 * SC) // P
                z_d = dscratch.tile([1, 2 * SC], f32, tag="zd", bufs=2,
                                    name=f"zd_{c}_{ibx}")
                zc = stream.tile([P, 2 * zw], f32, tag="zc", bufs=2,
                                 name=f"zc_{c}_{ibx}")
                rz_d = dscratch.tile([1, 2 * SC], f32, tag="rzd", bufs=2,
                                     name=f"rzd_{c}_{ibx}")
                rzb = stream.tile([DK, 2 * SC], f32, tag="rzb", bufs=2,
                                  name=f"rzb_{c}_{ibx}")
                stg = stream.tile([DK, SC], cdt, tag="stgB", bufs=2,
                                  name=f"stg_{c}_{ibx}")

                def z_chain(mul_engine, dq):
                    dq.dma_start(z_d[:], av_sb[DK:DK + 1, :],
                                 single_packet=True)
                    dq.dma_start(
                        zc[:, 0:zw],
                        z_d[:, :].rearrange("o (p x) -> (o p) x", p=P),
                        single_packet=True)
                    nc.vector.reciprocal(zc[:, zw:2 * zw], zc[:, 0:zw])
                    dq.dma_start(
                        rz_d[:, :].rearrange("o (p x) -> (o p) x", p=P),
                        zc[:, zw:2 * zw], single_packet=True)
                    dq.dma_start(
                        rzb[0:DK, :],
                        rz_d[:, :].to_broadcast((DK, 2 * SC)))
                    mul_engine.tensor_mul(on_c[c][0:DK, isl],
                                          av_sb[0:DK, 0:SC],
                                          rzb[0:DK, 0:SC])
                    mul_engine.tensor_mul(stg[0:DK, :],
                                          av_sb[0:DK, SC:2 * SC],
                                          rzb[0:DK, SC:2 * SC])
                    dq.dma_start(on_c[c][DK:P, isl], stg[0:DK, :])

                if last:
                    # end-of-kernel chain rides the scalar DGE queue: its
                    # DMA engines have no pending bulk stores, and the
                    # ACT queue has no latency-critical work left
                    z_chain(nc.vector, nc.scalar)
                else:
                    z_chain(nc.gpsimd, nc.sync)

            def emit_o_group(i, np_idx):
                # one psum tile covers output row-tiles n0=2*np_idx, n0+1
                # (shares the "sc" ring - O groups are rare enough that the
                # scores lookahead barely notices)
                n0 = 2 * np_idx
                pso = psum.tile([P, 2 * SC], f32, tag="sc", bufs=3,
                                name=f"ps_o_{i}_{np_idx}")
                csl = slice(i * SC, (i + 1) * SC)
                for sub in range(2):
                    n = n0 + sub
                    for c in range(HC):
                        nc.tensor.matmul(
                            pso[:, sub * SC:(sub + 1) * SC],
                            lhsT=wo_sb[:, c, n * P:(n + 1) * P],
                            rhs=on_c[c][:, csl],
                            start=(c == 0), stop=(c == HC - 1))
                stg = stream.tile([P, 2 * SC], cdt, tag="ostg", bufs=3,
                                  name=f"ostg_{i}_{np_idx}")
                if np_idx % 2 == 0:
                    nc.scalar.copy(stg[:], pso[:])
                else:
                    nc.vector.tensor_copy(stg[:], pso[:])
                nc.sync.dma_start(
                    outT[n0 * P:(n0 + 2) * P, csl].rearrange(
                        "(a p) s -> p a s", a=2),
                    stg[:].rearrange("p (a s) -> p a s", a=2))

            # Flat software pipeline over all (i-block, pair, jt) steps.
            steps = [(c, ibx, jt)
                     for ibx in range(NIB)
                     for c in range(HC)
                     for jt in range(JT)]
            e_ts = {}
            av_tiles = {}
            o_pending = []           # (chunk_idx, ready_group)
            o_np = 0                 # groups already emitted for head chunk
            for g0 in range(0, len(steps) + LAG, 2):
              # V-proj quads feed the first block's AV just in time
              if g0 in (0, 4, 8, 12):
                  vproj_quad(g0 // 4)
              for g in (g0, g0 + 1):
                if g < len(steps):
                    c, ibx, jt = steps[g]
                    isl = slice(ibx * ib, ibx * ib + ib)
                    sct = psum.tile([P, 2 * SC], f32, tag="sc",
                                    bufs=3, name=f"sc_{c}_{ibx}_{jt}")
                    jsl = slice(jt * P, (jt + 1) * P)
                    nc.tensor.matmul(
                        sct[:, 0:SC],
                        lhsT=kt[c][0:DK, jsl],
                        rhs=qt[c][0:DK, isl],
                        start=True, stop=True)
                    nc.tensor.matmul(
                        sct[:, SC:2 * SC],
                        lhsT=kt[c][DK:P, jsl],
                        rhs=qt[c][DK:P, isl],
                        start=True, stop=True)
                    et = stream.tile([P, 2 * SC], cdt, tag="e",
                                     bufs=6, name=f"e_{c}_{ibx}_{jt}")
                    if jt % 2 == 0 or jt == JT - 1 or DBG_EXP_ACT:
                        nc.scalar.activation(et[:], sct[:], Exp,
                                             bias=0.0, scale=scale)
                    else:
                        nc.vector.tensor_scalar(
                            et[:].bitcast(i16), sct[:],
                            SCH_A, SCH_B, Mult, Add)
                    e_ts[g] = et
              for g in (g0, g0 + 1):
                if g >= LAG and g < len(steps) + LAG:
                    gp = g - LAG
                    c, ibx, pj = steps[gp]
                    if pj == 0:
                        av_tiles[(c, ibx)] = psum.tile(
                            [P, 2 * SC], f32, tag="av", bufs=1,
                            name=f"av_{c}_{ibx}")
                    av = av_tiles[(c, ibx)]
                    et = e_ts.pop(gp)
                    st, sp = (pj == 0), (pj == JT - 1)
                    for h in range(HPC):
                        nc.tensor.matmul(
                            av[0:DK + 1, h * SC:(h + 1) * SC],
                            lhsT=v_c[c][:, pj, h, :],
                            rhs=et[:, h * SC:(h + 1) * SC],
                            start=st, stop=sp)
                    if pj == JT - 1:
                        emit_norm(c, ibx, av)
                        if c == HC - 1:
                            # the norm chain (4 DMA hops + gpsimd mults)
                            # takes ~8 steps; don't let the in-order PE
                            # queue reach the O matmuls before it's done
                            o_pending.append([ibx, g0 // 2 + 8])
              # interleave one output-projection psum group every other
              # pair-group
              if (o_pending and o_pending[0][1] <= g0 // 2
                      and (g0 // 2) % 2 == 0):
                  emit_o_group(o_pending[0][0], o_np)
                  o_np += 1
                  if o_np == NOUT // 2:
                      o_pending.pop(0)
                      o_np = 0
            # drain remaining chunks (the last i-block's)
            while o_pending:
                emit_o_group(o_pending[0][0], o_np)
                o_np += 1
                if o_np == NOUT // 2:
                    o_pending.pop(0)
                    o_np = 0

    nc.finalize()
    return nc


def make_in_maps(query, key, value, Wq, bq, Wk, bk, Wv, bv, Wo, bo):
    """Shard full inputs into the 8 per-core input dicts."""
    import ml_dtypes
    f = lambda a: np.ascontiguousarray(np.asarray(a, dtype=np.float32))
    HC = DH // P
    query, key, value = f(query), f(key), f(value)
    Wq, Wk, Wv, Wo = f(Wq), f(Wk), f(Wv), f(Wo)
    bq, bk, bv = f(bq), f(bk), f(bv)
    cvt = lambda a: np.ascontiguousarray(a.astype(ml_dtypes.bfloat16))
    in_maps = []
    for core in range(N_CORES):
        b, g = core // GROUPS, core % GROUPS
        sl = slice(g * DH, (g + 1) * DH)
        in_maps.append({
            "qT": cvt(query[b].T),
            "kTd": cvt(key[b].T),
            "vT": cvt(value[b].T),
            "wq": cvt(Wq[:, sl]),
            "wk": cvt(Wk[:, sl]),
            "wv": cvt(Wv[:, sl]),
            "wo": cvt(Wo[sl, :]),
            "bq": np.ascontiguousarray(bq[sl].reshape(HC, P).T),
            "bk": np.ascontiguousarray(bk[sl].reshape(HC, P).T),
        })
    return in_maps


# test hooks (ignored by the harness)
TRACE = False
LAST_RESULT = None
DTYPE = "bf16"
_NC_CACHE = {}


def kernel(query, key, value, Wq, bq, Wk, bk, Wv, bv, Wo, bo):
    global LAST_RESULT
    from concourse.bass_utils import run_bass_kernel_spmd

    if "nc" not in _NC_CACHE:
        _NC_CACHE["nc"] = build_nc()
    nc = _NC_CACHE["nc"]

    in_maps = make_in_maps(query, key, value, Wq, bq, Wk, bk, Wv, bv, Wo, bo)
    kwargs = {}
    if TRACE:
        kwargs = dict(trace=True, trace_cores=[0])
    res = run_bass_kernel_spmd(nc, in_maps, core_ids=list(range(N_CORES)),
                               **kwargs)
    LAST_RESULT = res

    out = np.zeros((B, S, D), np.float32)
    for core in range(N_CORES):
        b = core // GROUPS
        out[b] += res.results[core]["outT"].T.astype(np.float32)
    # bv contributes exactly bv @ Wo to every position (attention rows
    # sum to one), so it is folded here instead of on-device
    out += np.asarray(bo, dtype=np.float32) + (
        np.asarray(bv, np.float32) @ np.asarray(Wo, np.float32))
    return out


# revision 27
# speedup vs baseline: 1.0806x; 1.0806x over previous
"""Multi-head attention, tensor-parallel across 8 Trainium2 NeuronCores.

Sharding: core = (batch b, head-group g), g covering 4 heads (256 dh).
Within a core heads are processed as PAIRS: head A of a pair lives on
SBUF partitions 0-63, head B on 64-127, so the two K=64 scores matmuls
run concurrently on 64x128-mode array tiles T0/T8.

The attention is one flat software pipeline over (i-block, pair, j-tile)
steps batched two at a time (scores+exp of steps g, g+1 then the AV
matmuls of steps g-2, g-1), which keeps the exp pipeline primed across
i-block boundaries.  The output projection is interleaved into the
attention stream: chunk i's matmuls are emitted a few steps into block
i+1 (by which time both head-pairs of block i are normalized), filling
PE bubbles instead of running as a serial tail phase.

exp is split between ScalarE (exact Exp) and VectorE (a round-to-
nearest int16 Schraudolph in one tensor_scalar): bf16bits(exp(x)) ~
round(x*A + B) (fp32->int16 convert verified round-to-nearest on HW;
tile is bitcast to bf16 when consumed). Its ratio error is calibrated
mean-one, and softmax normalization cancels the common mode, ~1.3%.

Z comes from a ones column appended to V (AV psum row 64).  The Z rows
are reciprocal'd in-lane (1-lane ACT/DVE ops, no DRAM round trips),
broadcast across partitions by GPSIMD partition_broadcast, and GPSIMD
does the normalize mults.  Head B's normalized block is staged and
DMA'd to partitions 64-127 on the scalar DGE queue so it never queues
behind bulk output stores.  The last block runs its whole chain on
ACT/DVE at high priority to minimize the end-of-kernel latency.

DMA uses both hardware DGE queues: weights + latency-critical small
transfers on the Activation queue, bulk streams + output stores on SP.

Host: shards inputs, sums the 4 head-group partials per batch, adds bo.
"""

import os
import numpy as np

DBG_EXP_ACT = os.environ.get("DBG_EXP_ACT", "0") == "1"

B, S, D, H = 2, 2048, 1024, 16
DK = D // H              # 64 head dim
N_CORES = 8
GROUPS = N_CORES // B    # 4 head-groups
DH = D // GROUPS         # 256 head-dims per core (4 heads)
H_CORE = DH // DK        # 4 heads per core
SCALE = 1.0 / float(np.sqrt(DK))

P = 128                  # SBUF/PSUM partitions
SC = 512                 # matmul moving-dim chunk
IB = 512                 # flash i-block
LOG2E = float(np.log2(np.e))
SCH_A = float(128.0 * SCALE * LOG2E)       # schraudolph slope
SCH_B = float(127.0 * 128.0 - 7.35)        # schraudolph bias (mean-one)


def build_nc(S=S, D=D, DH=DH, DK=DK, scale=SCALE, ib=IB):
    import concourse.bacc as bacc
    import concourse.mybir as mybir
    import concourse.tile as tile

    f32 = mybir.dt.float32
    bf16 = mybir.dt.bfloat16
    i16 = mybir.dt.int16
    Exp = mybir.ActivationFunctionType.Exp
    Ident = mybir.ActivationFunctionType.Identity
    Recip = mybir.ActivationFunctionType.Reciprocal
    Mult = mybir.AluOpType.mult
    Add = mybir.AluOpType.add
    cdt = bf16

    KT = D // P                    # contraction tiles for projections (8)
    NSC = S // SC                  # s chunks (4)
    HC = DH // P                   # head pairs (2)
    HPC = P // DK                  # heads per pair (2)
    JT = S // P                    # j tiles (16)
    NIB = S // ib                  # i blocks (4)
    NOUT = D // P                  # output row chunks (8)
    LAG = 4                        # AV trails scores by LAG j-steps
                                   # (exp latency ~1.6us = 2.5 steps)

    nc = bacc.Bacc("TRN2", target_bir_lowering=False, debug=False)

    qT = nc.dram_tensor("qT", [D, S], cdt, kind="ExternalInput")
    kTd = nc.dram_tensor("kTd", [D, S], cdt, kind="ExternalInput")
    vT = nc.dram_tensor("vT", [D, S], cdt, kind="ExternalInput")
    wq = nc.dram_tensor("wq", [D, DH], cdt, kind="ExternalInput")
    wk = nc.dram_tensor("wk", [D, DH], cdt, kind="ExternalInput")
    wv = nc.dram_tensor("wv", [D, DH], cdt, kind="ExternalInput")
    wo = nc.dram_tensor("wo", [DH, D], cdt, kind="ExternalInput")
    bq = nc.dram_tensor("bq", [P, HC], f32, kind="ExternalInput")
    bk = nc.dram_tensor("bk", [P, HC], f32, kind="ExternalInput")
    outT = nc.dram_tensor("outT", [D, S], cdt, kind="ExternalOutput")

    with tile.TileContext(nc) as tc:
        with (
            tc.tile_pool(name="const", bufs=1) as cpool,
            tc.tile_pool(name="pers", bufs=1) as pers,
            tc.tile_pool(name="stream", bufs=1) as stream,
            tc.tile_pool(name="psum", bufs=1, space="PSUM") as psum,
            tc.tile_pool(name="dscratch", bufs=1, space="DRAM") as dscratch,
        ):
            # ---- constants ----
            wq_sb = cpool.tile([P, KT, DH], cdt, name="wq_sb")
            wk_sb = cpool.tile([P, KT, DH], cdt, name="wk_sb")
            wv_sb = cpool.tile([P, KT, DH], cdt, name="wv_sb")
            wo_sb = cpool.tile([P, HC, D], cdt, name="wo_sb")
            bq_sb = cpool.tile([P, HC], f32, name="bq_sb")
            bk_sb = cpool.tile([P, HC], f32, name="bk_sb")

            # ---- persistent activations (head-pair layout) ----
            # qt/kt pair c: rows 0-63 = head 2c (dk dims), rows 64-127 =
            # head 2c+1. v pair c: rows = j within tile, + ones column.
            qt = [pers.tile([P, S], cdt, name=f"qt{c}") for c in range(HC)]
            kt = [pers.tile([P, S], cdt, name=f"kt{c}") for c in range(HC)]
            v_c = [pers.tile([P, JT, HPC, DK + 1], cdt, name=f"v{c}")
                   for c in range(HC)]
            on_c = [pers.tile([P, S], cdt, name=f"on{c}") for c in range(HC)]

            for c in range(HC):
                nc.vector.memset(v_c[c][:, :, :, DK:DK + 1], 1.0)

            # ---- weights on the Activation DGE queue (parallel with the
            # bulk input streams on the SP queue) ----
            nc.scalar.dma_start(wq_sb[:],
                                wq[:, :].rearrange("(ko p) n -> p ko n", p=P))
            nc.scalar.dma_start(bq_sb[:], bq[:, :])
            nc.scalar.dma_start(wk_sb[:],
                                wk[:, :].rearrange("(ko p) n -> p ko n", p=P))
            nc.scalar.dma_start(bk_sb[:], bk[:, :])
            nc.scalar.dma_start(wv_sb[:],
                                wv[:, :].rearrange("(ko p) n -> p ko n", p=P))
            nc.scalar.dma_start(wo_sb[:],
                                wo[:, :].rearrange("(c p) n -> p c n", p=P))

            # ---- PE warmup: keep HAM busy during the initial input DMA
            # wait so the first real matmuls run at full clock ----
            wmp = psum.tile([P, 2 * SC], f32, tag="av", bufs=1, name="warm")
            wsrc = cpool.tile([P, DK], cdt, name="wsrc")
            nc.vector.memset(wsrc[:], 1.0)
            for w in range(64):
                nc.tensor.matmul(wmp[0:HPC, 0:DK], lhsT=wsrc[:, 0:HPC],
                                 rhs=wsrc[:, 0:DK], start=True, stop=True)

            # ---- bulk input streams, all on the Sync DGE queue (the
            # scalar queue only sustains ~1/3 the bandwidth).  Load order
            # is completion-criticality order: q's first SC chunk
            # (unblocks the PE), then all of k (gates attention start),
            # then v (needed LAG steps into attention), then the rest of
            # q (si1-3 are consumed by later i-blocks). ----
            def sb_in(src):
                return stream.tile([P, KT, S], cdt, tag="big_in", bufs=3,
                                   name=f"bi_{src.name}")

            def load_chunk(bt, src, lo, hi):
                for kti in range(KT):
                    nc.sync.dma_start(bt[:, kti, lo:hi],
                                      src[kti * P:(kti + 1) * P, lo:hi])

            qin, kin, vin = sb_in(qT), sb_in(kTd), sb_in(vT)
            load_chunk(qin, qT, 0, SC)
            load_chunk(kin, kTd, 0, S)
            load_chunk(vin, vT, 0, S)
            load_chunk(qin, qT, SC, S)

            def qk_proj_si(bt, w_sb, b_sb, dst, si):
                ps = psum.tile([P, 2 * SC], f32, tag="sc", bufs=3,
                               name=f"ps_{dst[0].name}_{si}")
                ssl = slice(si * SC, (si + 1) * SC)
                for c in range(HC):
                    for kti in range(KT):
                        nc.tensor.matmul(
                            ps[:, c * SC:(c + 1) * SC],
                            lhsT=w_sb[:, kti, c * P:(c + 1) * P],
                            rhs=bt[:, kti, ssl],
                            start=(kti == 0), stop=(kti == KT - 1))
                # evac + bias: head-pair chunk c goes straight to dst[c]
                nc.vector.tensor_add(
                    dst[0][:, ssl], ps[:, 0:SC],
                    b_sb[:, 0:1].to_broadcast((P, SC)))
                nc.scalar.activation(
                    dst[1][:, ssl], ps[:, SC:2 * SC], Ident,
                    bias=b_sb[:, 1:2], scale=1.0)

            qk_proj_si(qin, wq_sb, bq_sb, qt, 0)
            for si in range(NSC):
                qk_proj_si(kin, wk_sb, bk_sb, kt, si)

            # ---- V projection (natural [j, dh]), emitted interleaved
            # into early attention as quads of 4 j-tiles sharing one
            # sc-ring psum tile, so exp work starts right after K-proj
            # instead of behind a serial V phase.  bv is folded into the
            # host-side bias add (bv @ Wo), so evacs are plain copies. ----
            def vproj_quad(qd):
                ps = psum.tile([P, 2 * SC], f32, tag="sc", bufs=3,
                               name=f"ps_v_{qd}")
                for sub in range(4):
                    jt_idx = qd * 4 + sub
                    jsl = slice(jt_idx * P, (jt_idx + 1) * P)
                    for kti in range(KT):
                        nc.tensor.matmul(
                            ps[:, sub * DH:(sub + 1) * DH],
                            lhsT=vin[:, kti, jsl],
                            rhs=wv_sb[:, kti, :],
                            start=(kti == 0), stop=(kti == KT - 1))
                for sub in range(4):
                    jt_idx = qd * 4 + sub
                    for c in range(HC):
                        src_ap = ps[:, sub * DH + c * P:
                                    sub * DH + (c + 1) * P].rearrange(
                            "p (h d) -> p h d", d=DK)
                        dst_ap = v_c[c][:, jt_idx, :, 0:DK]
                        if (sub + c) % 2 == 0:
                            nc.vector.tensor_copy(dst_ap, src_ap)
                        else:
                            nc.scalar.copy(dst_ap, src_ap)

            # ---- attention (flash over j; head pairs on T0/T8) with the
            # output projection interleaved chunk-by-chunk ----
            def emit_norm(c, ibx, av):
                last = (ibx == NIB - 1 and c == HC - 1)
                isl = slice(ibx * ib, ibx * ib + ib)
                av_sb = stream.tile([P, 2 * SC], f32, tag="avsb", bufs=2,
                                    name=f"avsb_{c}_{ibx}")
                # evacuate both halves (frees the psum bank; av bufs=1)
                with tc.high_priority():
                    nc.scalar.copy(av_sb[0:DK + 1, 0:SC],
                                   av[0:DK + 1, 0:SC])
                    nc.vector.tensor_copy(av_sb[0:DK + 1, SC:2 * SC],
                                          av[0:DK + 1, SC:2 * SC])
                # Z rows DMA through DRAM reshaped [128, 8] for a cheap
                # all-lane reciprocal (1-lane DVE ops are ~6.5us - never).
                zw = (2 * SC) // P
                z_d = dscratch.tile([1, 2 * SC], f32, tag="zd", bufs=2,
                                    name=f"zd_{c}_{ibx}")
                zc = stream.tile([P, 2 * zw], f32, tag="zc", bufs=2,
                                 name=f"zc_{c}_{ibx}")
                rz_d = dscratch.tile([1, 2 * SC], f32, tag="rzd", bufs=2,
                                     name=f"rzd_{c}_{ibx}")
                rzb = stream.tile([DK, 2 * SC], f32, tag="rzb", bufs=2,
                                  name=f"rzb_{c}_{ibx}")
                stg = stream.tile([DK, SC], cdt, tag="stgB", bufs=2,
                                  name=f"stg_{c}_{ibx}")

                def z_chain(mul_engine, dq):
                    dq.dma_start(z_d[:], av_sb[DK:DK + 1, :],
                                 single_packet=True)
                    dq.dma_start(
                        zc[:, 0:zw],
                        z_d[:, :].rearrange("o (p x) -> (o p) x", p=P),
                        single_packet=True)
                    nc.vector.reciprocal(zc[:, zw:2 * zw], zc[:, 0:zw])
                    dq.dma_start(
                        rz_d[:, :].rearrange("o (p x) -> (o p) x", p=P),
                        zc[:, zw:2 * zw], single_packet=True)
                    dq.dma_start(
                        rzb[0:DK, :],
                        rz_d[:, :].to_broadcast((DK, 2 * SC)))
                    mul_engine.tensor_mul(on_c[c][0:DK, isl],
                                          av_sb[0:DK, 0:SC],
                                          rzb[0:DK, 0:SC])
                    mul_engine.tensor_mul(stg[0:DK, :],
                                          av_sb[0:DK, SC:2 * SC],
                                          rzb[0:DK, SC:2 * SC])
                    dq.dma_start(on_c[c][DK:P, isl], stg[0:DK, :])

                if last:
                    # end-of-kernel chain rides the scalar DGE queue: its
                    # DMA engines have no pending bulk stores, and the
                    # ACT queue has no latency-critical work left
                    z_chain(nc.vector, nc.scalar)
                else:
                    z_chain(nc.gpsimd, nc.sync)

            def emit_o_group(i, np_idx):
                # one psum tile covers output row-tiles n0=2*np_idx, n0+1
                # (shares the "sc" ring - O groups are rare enough that the
                # scores lookahead barely notices)
                n0 = 2 * np_idx
                pso = psum.tile([P, 2 * SC], f32, tag="sc", bufs=3,
                                name=f"ps_o_{i}_{np_idx}")
                csl = slice(i * SC, (i + 1) * SC)
                for sub in range(2):
                    n = n0 + sub
                    for c in range(HC):
                        nc.tensor.matmul(
                            pso[:, sub * SC:(sub + 1) * SC],
                            lhsT=wo_sb[:, c, n * P:(n + 1) * P],
                            rhs=on_c[c][:, csl],
                            start=(c == 0), stop=(c == HC - 1))
                stg = stream.tile([P, 2 * SC], cdt, tag="ostg", bufs=3,
                                  name=f"ostg_{i}_{np_idx}")
                if np_idx % 2 == 0:
                    nc.scalar.copy(stg[:], pso[:])
                else:
                    nc.vector.tensor_copy(stg[:], pso[:])
                nc.sync.dma_start(
                    outT[n0 * P:(n0 + 2) * P, csl].rearrange(
                        "(a p) s -> p a s", a=2),
                    stg[:].rearrange("p (a s) -> p a s", a=2))

            # Flat software pipeline over all (i-block, pair, jt) steps.
            steps = [(c, ibx, jt)
                     for ibx in range(NIB)
                     for c in range(HC)
                     for jt in range(JT)]
            e_ts = {}
            av_tiles = {}
            o_pending = []           # (chunk_idx, ready_group)
            o_np = 0                 # groups already emitted for head chunk
            for g0 in range(0, len(steps) + LAG, 2):
              # V-proj quads feed the first block's AV just in time
              if g0 in (0, 4, 8, 12):
                  vproj_quad(g0 // 4)
              # lazy Q-proj: qt si1-3 arrive long before the i-blocks
              # that read them (ibx reads si=ibx)
              if g0 in (16, 48, 80):
                  qk_proj_si(qin, wq_sb, bq_sb, qt, g0 // 32 + 1)
              for g in (g0, g0 + 1):
                if g < len(steps):
                    c, ibx, jt = steps[g]
                    isl = slice(ibx * ib, ibx * ib + ib)
                    sct = psum.tile([P, 2 * SC], f32, tag="sc",
                                    bufs=3, name=f"sc_{c}_{ibx}_{jt}")
                    jsl = slice(jt * P, (jt + 1) * P)
                    nc.tensor.matmul(
                        sct[:, 0:SC],
                        lhsT=kt[c][0:DK, jsl],
                        rhs=qt[c][0:DK, isl],
                        start=True, stop=True)
                    nc.tensor.matmul(
                        sct[:, SC:2 * SC],
                        lhsT=kt[c][DK:P, jsl],
                        rhs=qt[c][DK:P, isl],
                        start=True, stop=True)
                    et = stream.tile([P, 2 * SC], cdt, tag="e",
                                     bufs=8, name=f"e_{c}_{ibx}_{jt}")
                    if jt % 2 == 0 or jt == JT - 1 or DBG_EXP_ACT:
                        nc.scalar.activation(et[:], sct[:], Exp,
                                             bias=0.0, scale=scale)
                    else:
                        nc.vector.tensor_scalar(
                            et[:].bitcast(i16), sct[:],
                            SCH_A, SCH_B, Mult, Add)
                    e_ts[g] = et
              for g in (g0, g0 + 1):
                if g >= LAG and g < len(steps) + LAG:
                    gp = g - LAG
                    c, ibx, pj = steps[gp]
                    if pj == 0:
                        av_tiles[(c, ibx)] = psum.tile(
                            [P, 2 * SC], f32, tag="av", bufs=1,
                            name=f"av_{c}_{ibx}")
                    av = av_tiles[(c, ibx)]
                    et = e_ts.pop(gp)
                    st, sp = (pj == 0), (pj == JT - 1)
                    for h in range(HPC):
                        nc.tensor.matmul(
                            av[0:DK + 1, h * SC:(h + 1) * SC],
                            lhsT=v_c[c][:, pj, h, :],
                            rhs=et[:, h * SC:(h + 1) * SC],
                            start=st, stop=sp)
                    if pj == JT - 1:
                        emit_norm(c, ibx, av)
                        if c == HC - 1:
                            # the norm chain (4 DMA hops + gpsimd mults)
                            # takes ~8 steps; don't let the in-order PE
                            # queue reach the O matmuls before it's done
                            o_pending.append([ibx, g0 // 2 + 8])
              # interleave one output-projection psum group every other
              # pair-group
              if (o_pending and o_pending[0][1] <= g0 // 2
                      and (g0 // 2) % 2 == 0):
                  emit_o_group(o_pending[0][0], o_np)
                  o_np += 1
                  if o_np == NOUT // 2:
                      o_pending.pop(0)
                      o_np = 0
            # drain remaining chunks (the last i-block's)
            while o_pending:
                emit_o_group(o_pending[0][0], o_np)
                o_np += 1
                if o_np == NOUT // 2:
                    o_pending.pop(0)
                    o_np = 0

    nc.finalize()
    return nc


def make_in_maps(query, key, value, Wq, bq, Wk, bk, Wv, bv, Wo, bo):
    """Shard full inputs into the 8 per-core input dicts."""
    import ml_dtypes
    f = lambda a: np.ascontiguousarray(np.asarray(a, dtype=np.float32))
    HC = DH // P
    query, key, value = f(query), f(key), f(value)
    Wq, Wk, Wv, Wo = f(Wq), f(Wk), f(Wv), f(Wo)
    bq, bk, bv = f(bq), f(bk), f(bv)
    cvt = lambda a: np.ascontiguousarray(a.astype(ml_dtypes.bfloat16))
    in_maps = []
    for core in range(N_CORES):
        b, g = core // GROUPS, core % GROUPS
        sl = slice(g * DH, (g + 1) * DH)
        in_maps.append({
            "qT": cvt(query[b].T),
            "kTd": cvt(key[b].T),
            "vT": cvt(value[b].T),
            "wq": cvt(Wq[:, sl]),
            "wk": cvt(Wk[:, sl]),
            "wv": cvt(Wv[:, sl]),
            "wo": cvt(Wo[sl, :]),
            "bq": np.ascontiguousarray(bq[sl].reshape(HC, P).T),
            "bk": np.ascontiguousarray(bk[sl].reshape(HC, P).T),
        })
    return in_maps


# test hooks (ignored by the harness)
TRACE = False
LAST_RESULT = None
DTYPE = "bf16"
_NC_CACHE = {}


def kernel(query, key, value, Wq, bq, Wk, bk, Wv, bv, Wo, bo):
    global LAST_RESULT
    from concourse.bass_utils import run_bass_kernel_spmd

    if "nc" not in _NC_CACHE:
        _NC_CACHE["nc"] = build_nc()
    nc = _NC_CACHE["nc"]

    in_maps = make_in_maps(query, key, value, Wq, bq, Wk, bk, Wv, bv, Wo, bo)
    kwargs = {}
    if TRACE:
        kwargs = dict(trace=True, trace_cores=[0])
    res = run_bass_kernel_spmd(nc, in_maps, core_ids=list(range(N_CORES)),
                               **kwargs)
    LAST_RESULT = res

    out = np.zeros((B, S, D), np.float32)
    for core in range(N_CORES):
        b = core // GROUPS
        out[b] += res.results[core]["outT"].T.astype(np.float32)
    # bv contributes exactly bv @ Wo to every position (attention rows
    # sum to one), so it is folded here instead of on-device
    out += np.asarray(bo, dtype=np.float32) + (
        np.asarray(bv, np.float32) @ np.asarray(Wo, np.float32))
    return out
